# revision 73
# baseline (speedup 1.0000x reference)
"""Trainium2 Bass kernel for nn_BipartiteGNN (2x GCNConv + mean-pool + MLP head).

Strategy (single NeuronCore):
  - The end-to-end dispatch path to the axon-tunneled device is latency-bound
    (~40-80 ms of network RTT regardless of kernel); multi-core meshes pay
    ~2 ms extra client-side dispatch per exec with no latency benefit, and
    the whole GNN is a few ms of device work.  So: run everything on core 0,
    no collectives.
  - Nodes are permuted and bin-packed into 392 tiles x 128 nodes (balanced by
    in-degree).  Fused embed+dense1 streams x in 512-node chunks, computes
    hT = relu(We^T x^T + be) feature-major in SBUF, immediately applies W1
    and the dinv = rsqrt(deg+1) row scale, and writes fp8(e4m3) rows into a
    DRAM gather table [50176, 256] (tableA).  fp8 tables halve the gather
    bytes; emulation puts the end-to-end error at ~3e-4 (tolerance 2e-2).
  - Message gather via dma_gather (int16 idx; table split at row 25088 --
    balancing the two SWDGE queues -- into low/high passes).  The full idx
    streams are preloaded into SBUF once (both convs share them), gather
    chunks are 16 blocks with 6 prefetch buffers, and each tile's edge
    segment is sorted by source row so the descriptors walk HBM in ascending
    order.  Scatter-add via PE matmuls: for each 128-edge block,
    psum[tile] += S^T @ G, where S[e, n] = w_e * (dst_rel_e == n) is shipped
    as a host-packed fp8 input (one-hot placement of verbatim edge weights),
    loaded 4 tiles per DMA on the SP engine.
  - Epilogue h' = relu(dinv*(psum + slab) + b); slab (= dinv*t, reloaded from
    the table) supplies the self-loop term via an identity matmul; the dinv
    scale and bias-add run on DVE, relu on ACT.
  - conv0's epilogue transposes h1 through PSUM and immediately applies W2 +
    dinv (fused dense2), writing tableB; conv1 gathers from tableB and its
    epilogue feeds the mean-pool indicator matmuls directly, then the small
    MLP head runs feature-major.
All host-side work is index/layout preprocessing only (permutations, one-hot
placement of verbatim input values, padding); all arithmetic on tensor values
happens on device.
"""
import sys
import heapq
import numpy as np

sys.path.insert(0, "/opt/trn_rl_repo")

import concourse.bacc as bacc
import concourse.bass as bass
import concourse.tile as tile
from concourse import mybir
from concourse import bass_utils

# problem constants (hardcoded per harness contract)
N = 50000
E = 400000
G = 128
H = 256
FIN = 4
GF = 16
A = 64

NT = 392              # node tiles (128 nodes each)
TPC = NT * 128        # 50176 padded nodes
NPAD = TPC
TSPLIT = 25088        # gather-index split row (balances the two SWDGE queues;
                      # both halves must stay < 32768 for int16 indices)
CH = 16               # gather chunk, in 128-edge blocks
CH_HI = 16            # high-stream gather chunk
DG = 8                # dense-phase tile group (batched DMA)
EG = 512              # embed chunk (nodes)
PG = 8                # pool-indicator tile group

f16 = mybir.dt.float16
f32 = mybir.dt.float32
i16 = mybir.dt.int16


# ----------------------------------------------------------------------------
# host-side preprocessing (pure index/layout work)
# ----------------------------------------------------------------------------

def _greedy_pack(node_ids, keys, bin_ids):
    """Greedily pack node_ids (sorted desc by key priority) into bins of 128
    slots, minimizing the lexicographic (primary, secondary) load key.
    keys: [n, 2] per-node load contributions. Returns {node: (bin, slot)}."""
    heap = [(0.0, 0.0, b) for b in bin_ids]
    heapq.heapify(heap)
    slots_used = {b: 0 for b in bin_ids}
    out = {}
    for i, node in enumerate(node_ids):
        pend = []
        while True:
            l0, l1, b = heapq.heappop(heap)
            if slots_used[b] < 128:
                break
            pend.append((l0, l1, b))
        for p in pend:
            heapq.heappush(heap, p)
        out[node] = (b, slots_used[b])
        slots_used[b] += 1
        heapq.heappush(heap, (l0 + float(keys[i, 0]), l1 + float(keys[i, 1]), b))
    return out


def _assign_nodes(dst, src):
    """Balanced assignment of nodes to 392 tile bins of 128 slots.
    Pass 1 balances total in-degree; pass 2 rebalances within the low/high
    regions (bins < TSPLIT//128 vs the rest) on (high-deg, low-deg) jointly,
    which preserves every edge's low/high classification while minimizing the
    per-tile block counts. Returns perm (old node id -> id in [0, NPAD))."""
    deg = np.bincount(dst, minlength=N)
    nbins = NT
    order = np.argsort(-deg, kind="stable")
    keys = np.stack([deg[order], deg[order]], 1)
    a1 = _greedy_pack(order, keys, list(range(nbins)))
    perm = np.empty(N, np.int64)
    for node, (b, s) in a1.items():
        perm[node] = b * 128 + s
    # pass 2: regional rebalance on (B=high, A=low) in-degree
    lowreg = TSPLIT // 128
    sperm = perm[src]
    hi_edge = sperm >= TSPLIT
    degA = np.bincount(dst[~hi_edge], minlength=N)
    degB = np.bincount(dst[hi_edge], minlength=N)
    perm2 = np.empty(N, np.int64)
    for region_bins, nodes in (
        (list(range(lowreg)), np.nonzero(perm[np.arange(N)] // 128 < lowreg)[0]),
        (list(range(lowreg, nbins)), np.nonzero(perm[np.arange(N)] // 128 >= lowreg)[0]),
    ):
        nb_ = degB[nodes]
        na_ = degA[nodes]
        o = np.argsort(-(nb_ * 1000 + na_), kind="stable")
        nodes_o = nodes[o]
        keys2 = np.stack([nb_[o], na_[o]], 1)
        a2 = _greedy_pack(nodes_o, keys2, region_bins)
        for node, (b, s) in a2.items():
            perm2[node] = b * 128 + s
    return perm2


def _pack_idx_stream(stream, ch_blocks):
    """Pack an int stream (len multiple of 128) into the dma_gather int16
    layout: per chunk of ch_blocks*128 idxs -> [16, n/16] cols, replicated to
    128 partitions. Returns [128, total/16] int16."""
    total = len(stream)
    cols = []
    per = ch_blocks * 128
    for st in range(0, total, per):
        chunk = stream[st:st + per]
        m = len(chunk)
        base = chunk.reshape(m // 16, 16).T.astype(np.int16)  # [16, m/16]
        cols.append(np.tile(base, (8, 1)))
    return np.concatenate(cols, axis=1)


def _prep(inputs, fp8=True):
    x = np.asarray(inputs["x"], np.float32)
    ei = np.asarray(inputs["edge_index"], np.int64)
    batch = np.asarray(inputs["batch"], np.int64)
    gfeat = np.asarray(inputs["global_features"], np.float32)
    ew = np.asarray(inputs["edge_weight"], np.float32)

    src, dst = ei[0], ei[1]
    perm = _assign_nodes(dst, src)

    sperm = perm[src]
    dperm = perm[dst]
    bins = dperm // 128          # tile id 0..391
    drel = (dperm % 128).astype(np.float32)
    low = sperm < TSPLIT

    nbins = NT
    # order edges by (bin, half), ascending src within each segment so the
    # gather descriptors walk HBM in ascending address order
    key = bins * 2 + (~low).astype(np.int64)
    eorder = np.lexsort((sperm, key))
    key_s = key[eorder]
    cnt = np.bincount(key_s, minlength=nbins * 2)
    offs = np.concatenate([[0], np.cumsum(cnt)])
    cnt_lo = cnt[0::2]
    cnt_hi = cnt[1::2]
    KA = max(1, int(np.ceil(cnt_lo.max() / 128)))
    KB = max(1, int(np.ceil(cnt_hi.max() / 128))) if cnt_hi.max() > 0 else 0
    B = KA + KB

    s_s = sperm[eorder]
    d_s = drel[eorder]
    w_s = ew[eorder]

    idx_lo = np.zeros((nbins, KA * 128), np.int64)
    idx_hi = np.zeros((nbins, max(KB, 1) * 128), np.int64)
    dr_all = np.zeros((nbins, B * 128), np.float32)
    w_all = np.zeros((nbins, B * 128), np.float32)
    for b in range(nbins):
        lo0, lo1 = offs[2 * b], offs[2 * b + 1]
        hi1 = offs[2 * b + 2]
        nl = lo1 - lo0
        nh = hi1 - lo1
        idx_lo[b, :nl] = s_s[lo0:lo1]
        dr_all[b, :nl] = d_s[lo0:lo1]
        w_all[b, :nl] = w_s[lo0:lo1]
        if KB:
            idx_hi[b, :nh] = s_s[lo1:hi1] - TSPLIT
            dr_all[b, KA * 128:KA * 128 + nh] = d_s[lo1:hi1]
            w_all[b, KA * 128:KA * 128 + nh] = w_s[lo1:hi1]

    # per-node padded weight lists for deg (over ALL in-edges)
    Dmax = max(1, int(np.bincount(dperm, minlength=NPAD).max()))
    nodesort = np.argsort(dperm, kind="stable")
    dp_s = dperm[nodesort]
    w_ns = ew[nodesort]
    noffs = np.concatenate([[0], np.cumsum(np.bincount(dp_s, minlength=NPAD))])
    pos = np.arange(len(dp_s)) - noffs[dp_s]
    wpad = np.zeros(NPAD * Dmax, np.float32)
    wpad[dp_s * Dmax + pos] = w_ns
    wpad = wpad.reshape(NPAD, Dmax)

    # graph of each permuted node; dummies -> -1
    gid = np.full(NPAD, -1, np.int64)
    gid[perm] = batch

    cntg = np.bincount(batch, minlength=G).astype(np.float32)
    recip = 1.0 / np.maximum(cntg, 1.0)

    # x rows in permuted order, transposed
    xp = np.zeros((NPAD, FIN), np.float32)
    xp[perm] = x

    # weights
    W_emb = np.asarray(inputs["W_emb"], np.float32)
    b_emb = np.asarray(inputs["b_emb"], np.float32)
    Wembc = np.stack([W_emb[:, :128], W_emb[:, 128:]], 1).astype(np.float16)
    bemb_col = np.stack([b_emb[:128], b_emb[128:]], 1).astype(np.float32)  # [128,2]

    def wchunks(Wm):  # [256,256] -> [128, 2, 256]
        Wm = np.asarray(Wm, np.float32)
        return np.stack([Wm[:128], Wm[128:]], 1).astype(np.float16)

    W1c = wchunks(inputs["W1"])
    W2c = wchunks(inputs["W2"])
    b1b = np.broadcast_to(np.asarray(inputs["b1"], np.float32)[None, :], (128, H)).astype(np.float16).copy()
    b2b = np.broadcast_to(np.asarray(inputs["b2"], np.float32)[None, :], (128, H)).astype(np.float16).copy()

    Wg = np.asarray(inputs["Wg"], np.float32)       # [16,256]
    Wg_l = np.stack([Wg[:, :128], Wg[:, 128:]], 1).astype(np.float16)   # [16,2,128]
    bg = np.asarray(inputs["bg"], np.float32)
    bg_col = np.stack([bg[:128], bg[128:]], 1)       # [128,2]
    Wh1 = np.asarray(inputs["Wh1"], np.float32)      # [512,256]
    Wh1_l = np.zeros((128, 4, 2, 128), np.float16)
    for k in range(4):
        for m in range(2):
            Wh1_l[:, k, m, :] = Wh1[k * 128:(k + 1) * 128, m * 128:(m + 1) * 128]
    bh1 = np.asarray(inputs["bh1"], np.float32)
    bh1_col = np.stack([bh1[:128], bh1[128:]], 1)    # [128,2]
    Wh2 = np.asarray(inputs["Wh2"], np.float32)      # [256,64]
    Wh2_l = np.stack([Wh2[:128], Wh2[128:]], 1).astype(np.float16)  # [128,2,64]
    bh2_col = np.asarray(inputs["bh2"], np.float32)[:, None]        # [64,1]
    gfT = gfeat.T.astype(np.float16)                 # [16,128]
    recipb = np.broadcast_to(recip[None, None, :], (128, 2, G)).astype(np.float32).copy()

    ilo = idx_lo.reshape(-1)
    ihi = idx_hi.reshape(-1)
    # S blocks: one-hot placement of verbatim edge weights.
    # S[slot, t*B+j, drel] = w for edge at (tile t, block j, slot)
    s_np = mybir.dt.np(mybir.dt.float8e4) if fp8 else np.float16
    S_host = np.zeros((128, NT * B, 128), s_np)
    dr_flat = dr_all.reshape(-1).astype(np.int64)   # [NT*B*128]
    w_flat = w_all.reshape(-1)
    blk_flat = np.repeat(np.arange(NT * B), 128)
    slot_flat = np.tile(np.arange(128), NT * B)
    mask = w_flat != 0.0
    S_host[slot_flat[mask], blk_flat[mask], dr_flat[mask]] = w_flat[mask]
    wp = wpad.reshape(NT, 128, Dmax).transpose(1, 0, 2)  # [128, NT, Dmax]
    gl = gid.reshape(NT, 128).T  # [128, NT]
    pind = np.zeros((128, NT, G), np.float16)
    pp, tt = np.nonzero(gl >= 0)
    pind[pp, tt, gl[pp, tt]] = 1.0
    xT5 = xp.T.astype(np.float16)  # [4, TPC]

    in_map = {
        "xT": xT5,
        "Wembc": Wembc, "bemb_col": bemb_col,
        "W1c": W1c, "W2c": W2c, "b1b": b1b, "b2b": b2b,
        "idx_lo": _pack_idx_stream(ilo, CH),
        "idx_hi": _pack_idx_stream(ihi, CH_HI) if KB else np.zeros((128, 8), np.int16),
        "S": S_host,
        "wpad": wp.astype(np.float32).copy(),
        "pind": pind,
        "Wg_l": Wg_l, "bg_col": bg_col.astype(np.float32).copy(),
        "Wh1_l": Wh1_l, "bh1_col": bh1_col.astype(np.float32).copy(),
        "Wh2_l": Wh2_l, "bh2_col": bh2_col.astype(np.float32).copy(),
        "gfT": gfT, "recipb": recipb,
    }
    meta = dict(KA=KA, KB=KB, B=B, Dmax=Dmax, fp8=fp8)
    return [in_map], meta


# ----------------------------------------------------------------------------
# device program (single core)
# ----------------------------------------------------------------------------

def _build(meta, single_queue=False, abl=frozenset()):
    KA, KB, B, Dmax = meta["KA"], meta["KB"], meta["B"], meta["Dmax"]
    hi_q = 0 if single_queue else 1
    nqueues = 4 if "q4" in abl else 2
    tf = mybir.dt.float8e4 if meta.get("fp8") else f16  # table/S/gather dtype
    nc = bacc.Bacc("TRN2", target_bir_lowering=False, debug=False,
                   num_devices=1, num_swdge_queues=nqueues)

    def din(name, shape, dt):
        return nc.dram_tensor(name, shape, dt, kind="ExternalInput")

    xT_d = din("xT", [FIN, TPC], f16)
    Wembc_d = din("Wembc", [FIN, 2, 128], f16)
    bemb_d = din("bemb_col", [128, 2], f32)
    W1c_d = din("W1c", [128, 2, H], f16)
    W2c_d = din("W2c", [128, 2, H], f16)
    b1b_d = din("b1b", [128, H], f16)
    b2b_d = din("b2b", [128, H], f16)
    idx_lo_d = din("idx_lo", [128, NT * KA * 8], i16)
    idx_hi_d = din("idx_hi", [128, (NT * KB * 8) if KB else 8], i16)
    S_d = din("S", [128, NT * B, 128], tf)
    wpad_d = din("wpad", [128, NT, Dmax], f32)
    pind_d = din("pind", [128, NT, G], f16)
    Wg_d = din("Wg_l", [GF, 2, 128], f16)
    bg_d = din("bg_col", [128, 2], f32)
    Wh1_d = din("Wh1_l", [128, 4, 2, 128], f16)
    bh1_d = din("bh1_col", [128, 2], f32)
    Wh2_d = din("Wh2_l", [128, 2, A], f16)
    bh2_d = din("bh2_col", [A, 1], f32)
    gfT_d = din("gfT", [GF, G], f16)
    recipb_d = din("recipb", [128, 2, G], f32)
    out_q = nc.dram_tensor("out_q", [A, G], f32, kind="ExternalOutput")

    with tile.TileContext(nc) as tc:
        with tc.tile_pool(name="const", bufs=1) as cst, \
             tc.tile_pool(name="work", bufs=2) as wk, \
             tc.tile_pool(name="sbuild", bufs=3) as sbp, \
             tc.tile_pool(name="gat", bufs=6) as gp, \
             tc.tile_pool(name="epi", bufs=4) as ep, \
             tc.tile_pool(name="psA", bufs=2, space="PSUM") as psA, \
             tc.tile_pool(name="psB", bufs=4, space="PSUM") as psB, \
             tc.tile_pool(name="psP", bufs=2, space="PSUM") as psP, \
             tc.tile_pool(name="dram", bufs=1, space="DRAM") as dr:
            # conv0's transpose PSUM shares the pool-accumulator pool (psP):
            # psP is live only during conv1, the transposes only during conv0
            psT = psP

            # ---- constants to SBUF
            def load(pool, dram, shape, dt, tag, eng=nc.scalar):
                t = pool.tile(shape, dt, tag=tag)
                eng.dma_start(out=t[:], in_=dram.ap())
                return t

            Wembc = load(cst, Wembc_d, [FIN, 2, 128], f16, "Wembc")
            bembc = load(cst, bemb_d, [128, 2], f32, "bembc")
            W1c = load(cst, W1c_d, [128, 2, H], f16, "W1c")
            W2c = load(cst, W2c_d, [128, 2, H], f16, "W2c")
            b1b = load(cst, b1b_d, [128, H], f16, "b1b")
            b2b = load(cst, b2b_d, [128, H], f16, "b2b")
            wpad = load(cst, wpad_d, [128, NT, Dmax], f32, "wpad")
            Wg_sb = load(cst, Wg_d, [GF, 2, 128], f16, "Wg")
            bg_sb = load(cst, bg_d, [128, 2], f32, "bg")
            Wh1_sb = load(cst, Wh1_d, [128, 4, 2, 128], f16, "Wh1")
            bh1_sb = load(cst, bh1_d, [128, 2], f32, "bh1")
            Wh2_sb = load(cst, Wh2_d, [128, 2, A], f16, "Wh2")
            bh2_sb = load(cst, bh2_d, [A, 1], f32, "bh2")
            gfT_sb = load(cst, gfT_d, [GF, G], f16, "gfT")
            recipb = load(cst, recipb_d, [128, 2, G], f32, "recipb")
            # full idx streams resident in SBUF (shared by both convs) so the
            # gather queues never wait on idx DMAs
            idx_lo_sb = load(cst, idx_lo_d, [128, NT * KA * 8], i16, "idxlo")
            idx_hi_sb = load(cst, idx_hi_d,
                             [128, (NT * KB * 8) if KB else 8], i16, "idxhi")

            from concourse.masks import make_identity
            ident = cst.tile([128, 128], tf, tag="ident")
            make_identity(nc, ident[:])
            dummy_g = None
            if "mm_dummy" in abl:
                dummy_g = cst.tile([128, H], tf, tag="dummy_g")
                nc.gpsimd.memset(dummy_g[:], 0.25)
            if tf is f16:
                ident16 = ident
            else:
                ident16 = cst.tile([128, 128], f16, tag="ident16")
                make_identity(nc, ident16[:])

            # ---- deg / dinv
            deg = cst.tile([128, NT], f32, tag="deg")
            nc.vector.tensor_reduce(deg[:], wpad[:], axis=mybir.AxisListType.X,
                                    op=mybir.AluOpType.add)
            sq = cst.tile([128, NT], f32, tag="sq")
            nc.scalar.activation(sq[:], deg[:],
                                 mybir.ActivationFunctionType.Sqrt, bias=1.0)
            dinv = cst.tile([128, NT], f32, tag="dinv")
            nc.vector.reciprocal(dinv[:], sq[:])



            # ---- DRAM gather tables (ping/pong per conv), split at TSPLIT so
            # each half is a separate tensor: the low-half gathers only depend
            # on the low-half writes (tiles < NTLO), letting each conv's
            # gather stream start while the previous phase is still writing
            # the high half.
            NTLO = TSPLIT // 128
            assert NTLO * 128 == TSPLIT and NTLO % 4 == 0
            tableA_lo = dr.tile([TSPLIT, H], tf, tag="tableA_lo")
            tableA_hi = dr.tile([NPAD - TSPLIT, H], tf, tag="tableA_hi")
            tableB_lo = dr.tile([TSPLIT, H], tf, tag="tableB_lo")
            tableB_hi = dr.tile([NPAD - TSPLIT, H], tf, tag="tableB_hi")
            tA_lo_t = tableA_lo[:].rearrange("(t p) f -> p t f", p=128)
            tA_hi_t = tableA_hi[:].rearrange("(t p) f -> p t f", p=128)
            tB_lo_t = tableB_lo[:].rearrange("(t p) f -> p t f", p=128)
            tB_hi_t = tableB_hi[:].rearrange("(t p) f -> p t f", p=128)

            def tview(lo_t, hi_t, t0, n):
                # per-tile-group view; groups are 4-aligned and NTLO % 4 == 0,
                # so a group never straddles the lo/hi boundary
                if t0 < NTLO:
                    return lo_t[:, t0:t0 + n, :]
                return hi_t[:, t0 - NTLO:t0 - NTLO + n, :]

            # ---- fused embed + dense1: tableA rows = dinv * (relu(x@We+be) @ W1)
            EGT = EG // 128   # tiles per embed group
            for g0 in range(0, TPC, EG):
                t0 = g0 // 128
                xch = wk.tile([FIN, EG], f16, tag="xch")
                nc.scalar.dma_start(out=xch[:], in_=xT_d.ap()[:, g0:g0 + EG])
                hTw = wk.tile([128, 2, EG], f16, tag="hTw")
                for k in range(2):
                    pa = psA.tile([128, EG], f32, tag="psA",
                                  name=f"psE{g0}_{k}")
                    if "no_dense" not in abl:
                        nc.tensor.matmul(out=pa[:], lhsT=Wembc[:, k, :],
                                         rhs=xch[:],
                                         start=True, stop=True)
                    nc.scalar.activation(hTw[:, k, :], pa[:],
                                         mybir.ActivationFunctionType.Relu,
                                         bias=bembc[:, k:k + 1])
                slab1w = wk.tile([128, EGT, H], tf, tag="slab1w")
                for j in range(EGT):
                    t = t0 + j
                    pa = psA.tile([128, H], f32, tag="psA")
                    if "no_dense" not in abl:
                        for k in range(2):
                            nc.tensor.matmul(
                                out=pa[:],
                                lhsT=hTw[:, k, j * 128:(j + 1) * 128],
                                rhs=W1c[:, k, :],
                                start=(k == 0), stop=(k == 1))
                    nc.scalar.activation(slab1w[:, j, :], pa[:],
                                         mybir.ActivationFunctionType.Copy,
                                         scale=dinv[:, t:t + 1])
                nc.sync.dma_start(out=tview(tA_lo_t, tA_hi_t, t0, EGT),
                                  in_=slab1w[:])

            # ---- two conv scatter phases
            # conv 0: gathers from tableA; epilogue h1 = relu(dinv*psum + b1),
            #         then transpose + dense2 writes tableB rows.
            # conv 1: gathers from tableB; epilogue h2 feeds pool matmuls.
            for conv in range(2):
                bb = b1b if conv == 0 else b2b
                lo_tbl = tableA_lo if conv == 0 else tableB_lo
                hi_tbl = tableA_hi if conv == 0 else tableB_hi
                lo_t = tA_lo_t if conv == 0 else tB_lo_t
                hi_t = tA_hi_t if conv == 0 else tB_hi_t

                nlow = NT * KA
                nhigh = NT * KB
                glow_tiles = {}
                ghigh_tiles = {}

                def ensure_gather(stream_blocks, bidx, tiles_map, idx_sb, tbl_view,
                                  tag, qn=0, ch=CH):
                    k = bidx // ch
                    if k in tiles_map:
                        return tiles_map[k]
                    nb = min(ch, stream_blocks - k * ch)
                    gt = gp.tile([128, ch, H], tf, tag=tag)
                    qn_eff = (qn * 2 + k % 2) if "q4" in abl else qn
                    g_nb = 1 if "tiny_gather" in abl else (
                        max(1, nb // 2) if "half_gather" in abl else nb)
                    nc.gpsimd.dma_gather(
                        out_ap=gt[:, :g_nb, :], in_ap=tbl_view,
                        idxs_ap=idx_sb[:, k * ch * 8:k * ch * 8 + nb * 8],
                        num_idxs=g_nb * 128,
                        num_idxs_reg=g_nb * 128, elem_size=H, queue_num=qn_eff,
                        single_packet=False)
                    tiles_map[k] = gt
                    return gt

                pool_ps = None
                if conv == 1:
                    pool_ps = [psP.tile([128, G], f32, tag="psP",
                                        name=f"pool_ps{i}") for i in range(2)]

                slab2w = None
                SB = 4   # tiles per batched S / slab-reload DMA
                S_t = None
                slabr = None
                for t in range(NT):
                    pb = psB.tile([128, H], f32, tag="psB")
                    if t % SB == 0:
                        sn = min(SB, NT - t)
                        S_t = sbp.tile([128, SB * B, 128], tf, tag="S")
                        s_n = 1 if "tiny_s" in abl else sn * B
                        nc.sync.dma_start(
                            out=S_t[:, :s_n, :],
                            in_=S_d.ap()[:, t * B:t * B + s_n, :])
                        slabr = wk.tile([128, SB, H], tf, tag="slabr")
                        nc.scalar.dma_start(out=slabr[:, :sn, :],
                                            in_=tview(lo_t, hi_t, t, sn))
                    so = (t % SB) * B
                    mm = 0
                    if "no_scatter_mm" not in abl:
                        for j in range(KA):
                            b = t * KA + j
                            gt = ensure_gather(nlow, b, glow_tiles, idx_lo_sb,
                                               lo_tbl[:], "glow", qn=0)
                            rhs_g = dummy_g[:] if dummy_g is not None \
                                else gt[:, b % CH, :]
                            nc.tensor.matmul(out=pb[:], lhsT=S_t[:, so + j, :],
                                             rhs=rhs_g,
                                             start=(mm == 0), stop=False)
                            mm += 1
                        for j in range(KB):
                            b = t * KB + j
                            gt = ensure_gather(nhigh, b, ghigh_tiles, idx_hi_sb,
                                               hi_tbl[:], "ghigh", qn=hi_q,
                                               ch=CH_HI)
                            rhs_g = dummy_g[:] if dummy_g is not None \
                                else gt[:, b % CH_HI, :]
                            nc.tensor.matmul(out=pb[:], lhsT=S_t[:, so + KA + j, :],
                                             rhs=rhs_g,
                                             start=(mm == 0), stop=False)
                            mm += 1
                    nc.tensor.matmul(out=pb[:], lhsT=ident[:],
                                     rhs=slabr[:, t % SB, :],
                                     start=(mm == 0), stop=True)
                    # epilogue: h' = relu(dinv*pb + b) (scale+bias on DVE)
                    tmp = ep.tile([128, H], f16, tag="tmp")
                    nc.vector.tensor_scalar(tmp[:], pb[:], dinv[:, t:t + 1],
                                            None, op0=mybir.AluOpType.mult)
                    hsc = ep.tile([128, H], f16, tag="hsc")
                    nc.vector.tensor_tensor(hsc[:], tmp[:], bb[:],
                                            op=mybir.AluOpType.add)
                    if conv == 0:
                        hre = ep.tile([128, H], f16, tag="hre")
                        nc.scalar.activation(hre[:], hsc[:],
                                             mybir.ActivationFunctionType.Relu)
                        # transpose h1 tile; fused dense2 -> tableB rows
                        pt = psT.tile([128, 2, 128], f16, tag="psP",
                                      name=f"ptC{t}")
                        if "no_dense" not in abl:
                            for k in range(2):
                                nc.tensor.transpose(
                                    pt[:, k, :], hre[:, k * 128:(k + 1) * 128],
                                    ident16[:])
                        hTw2 = ep.tile([128, 2, 128], f16, tag="hTw2")
                        nc.scalar.activation(hTw2[:], pt[:],
                                             mybir.ActivationFunctionType.Copy)
                        pa2 = psA.tile([128, H], f32, tag="psA")
                        if "no_dense" not in abl:
                            for k in range(2):
                                nc.tensor.matmul(
                                    out=pa2[:], lhsT=hTw2[:, k, :],
                                    rhs=W2c[:, k, :],
                                    start=(k == 0), stop=(k == 1))
                        if t % 4 == 0:
                            slab2w = wk.tile([128, 4, H], tf, tag="slab2w")
                        nc.scalar.activation(slab2w[:, t % 4, :], pa2[:],
                                             mybir.ActivationFunctionType.Copy,
                                             scale=dinv[:, t:t + 1])
                        if t % 4 == 3 or t == NT - 1:
                            tb0 = t - t % 4
                            nc.sync.dma_start(
                                out=tview(tB_lo_t, tB_hi_t, tb0, t - tb0 + 1),
                                in_=slab2w[:, :t % 4 + 1, :])
                    else:
                        h2sc = ep.tile([128, H], f16, tag="h2sc")
                        nc.scalar.activation(h2sc[:], hsc[:],
                                             mybir.ActivationFunctionType.Relu)
                        if t % PG == 0:
                            pind_t = wk.tile([128, PG, G], f16, tag="pind")
                            pn = min(PG, NT - t)
                            nc.sync.dma_start(
                                out=pind_t[:, :pn, :],
                                in_=pind_d.ap()[:, t:t + pn, :])
                        for m in range(2):
                            nc.tensor.matmul(
                                out=pool_ps[m][:],
                                lhsT=h2sc[:, m * 128:(m + 1) * 128],
                                rhs=pind_t[:, t % PG, :],
                                start=(t == 0), stop=(t == NT - 1))

            # ---- mean pool epilogue + head (single core, no collectives)
            poolT = wk.tile([128, 2, G], f32, tag="poolT")
            for m in range(2):
                nc.vector.tensor_copy(poolT[:, m, :], pool_ps[m][:])

            combT = wk.tile([128, 4, G], f16, tag="combT")
            nc.vector.tensor_tensor(combT[:, 0:2, :], poolT[:], recipb[:],
                                    op=mybir.AluOpType.mult)
            # global embed: relu(Wg^T @ gfT + bg)
            for m in range(2):
                pe = psP.tile([128, G], f32, tag="psP")
                nc.tensor.matmul(out=pe[:], lhsT=Wg_sb[:, m, :], rhs=gfT_sb[:],
                                 start=True, stop=True)
                nc.scalar.activation(combT[:, 2 + m, :], pe[:],
                                     mybir.ActivationFunctionType.Relu,
                                     bias=bg_sb[:, m:m + 1])
            # q1 = relu(Wh1^T @ comb + bh1)
            q1T = wk.tile([128, 2, G], f16, tag="q1T")
            for m in range(2):
                pq = psP.tile([128, G], f32, tag="psP")
                for k in range(4):
                    nc.tensor.matmul(out=pq[:], lhsT=Wh1_sb[:, k, m, :],
                                     rhs=combT[:, k, :],
                                     start=(k == 0), stop=(k == 3))
                nc.scalar.activation(q1T[:, m, :], pq[:],
                                     mybir.ActivationFunctionType.Relu,
                                     bias=bh1_sb[:, m:m + 1])
            # q = Wh2^T @ q1 + bh2
            pqf = psP.tile([A, G], f32, tag="psP")
            for k in range(2):
                nc.tensor.matmul(out=pqf[:], lhsT=Wh2_sb[:, k, :],
                                 rhs=q1T[:, k, :], start=(k == 0), stop=(k == 1))
            qT = wk.tile([A, G], f32, tag="qT")
            nc.scalar.activation(qT[:], pqf[:],
                                 mybir.ActivationFunctionType.Identity,
                                 bias=bh2_sb[:])
            nc.sync.dma_start(out=out_q.ap(), in_=qT[:])

    nc.compile()
    return nc


_CACHE = {}


def kernel(**inputs):
    in_maps, meta = _prep(inputs)
    key = (meta["KA"], meta["KB"], meta["Dmax"], meta["fp8"])
    if key not in _CACHE:
        _CACHE[key] = _build(meta)
    nc = _CACHE[key]
    res = bass_utils.run_bass_kernel_spmd(
        nc, in_maps, core_ids=[0], trace=False)
    q = res.results[0]["out_q"].T.astype(np.float32).copy()
    # stash for test harness reuse (timing)
    kernel._last = (nc, in_maps)
    return q

